# revision 12
# baseline (speedup 1.0000x reference)
"""Trainium2 Bass kernel for nn_CodedNet (roll -> binary mask -> unroll -> channel sum).

Math simplification: the forward roll by -ch, the 64x64 binary mask multiply,
and the backward roll by +ch collapse to

    out[b,i,w] = sum_ch x[b,i,w,ch] * mask32[(i-ch)%32, w%32]

where mask32 = sign(w_in).reshape(32,32)  (the 64x64 mask is a 2x2 tile of it).

v2 strategy ("scan"): fp16 datapath + fused multiply/segmented-reduce via
tensor_tensor_scan. With sigma[c] = m[c]*m[c-1] (0 at each 31-channel group
start) the recurrence

    S[c] = sigma[c]*S[c-1] + x[c]        (fp32 internal state)

satisfies S[30] = m[30] * sum_c m[c]*x[c], so one scan pass replaces the
multiply AND the reduce. A tiny strided multiply by m_end extracts the group
sums. Scans are split between DVE and GPSIMD so both stay under the DMA
roofline (~16.8 MB of HBM traffic per core in fp16).

Sharding: pure data parallel over batch (512 -> 64 per core on 8 cores).
"""

import sys

if "/opt/trn_rl_repo" not in sys.path:
    sys.path.insert(0, "/opt/trn_rl_repo")

import numpy as np

B, H, W, CH = 512, 64, 64, 31
N_CORES = 8
B_PER_CORE = B // N_CORES  # 64
BPT = 4  # batches per fused tile (2 pairs of 2)
N_TILES = B_PER_CORE // BPT  # 16
FREE = W * CH  # 1984

TRACE = False

_nc_cache: dict = {}


def _gp_half_scans(k: int) -> set:
    """Spread k of the 32 half-scans evenly across the tile sequence."""
    js = {int(round((i + 0.5) * 32 / k)) for i in range(k)} if k else set()
    return {(j // 2, j % 2) for j in js}


def _emit_scan(tc, x, sg, me, out, ext_eng="gpsimd", bufs=4, out_ring="scalar",
               in_ring="sync"):
    import concourse.mybir as mybir
    from concourse.alu_op_type import AluOpType

    nc = tc.nc
    f16 = mybir.dt.float16
    oring = getattr(nc, out_ring)
    iring = getattr(nc, in_ring)
    ext = getattr(nc, ext_eng)

    # tile t covers batches 4t..4t+3; partition = (b%2, i); halves g = 0,1
    xv = x.rearrange("(t g b) i w c -> t (b i) g (w c)", g=2, b=2)  # [16,128,2,1984]
    ov = out.rearrange("(t g b) i w -> t (b i) g w", g=2, b=2)  # [16,128,2,64]

    with (
        tc.tile_pool(name="const", bufs=1) as cpool,
        tc.tile_pool(name="xbuf", bufs=bufs) as xpool,
        tc.tile_pool(name="sbuf", bufs=bufs) as spool,
        tc.tile_pool(name="rbuf", bufs=4) as rpool,
        nc.allow_low_precision(reason="fp16 datapath; fp32 scan state"),
    ):
        sgt = cpool.tile([128, 2 * FREE], f16)
        oring.dma_start(out=sgt[:], in_=sg)
        met = cpool.tile([128, 2 * W], f16)
        oring.dma_start(out=met[:], in_=me)
        for t in range(N_TILES):
            xt = xpool.tile([128, 2 * FREE], f16)
            xtv = xt[:].rearrange("p (g f) -> p g f", g=2)
            iring.dma_start(out=xtv[:, 0], in_=xv[t, :, 0])
            iring.dma_start(out=xtv[:, 1], in_=xv[t, :, 1])
            sc = spool.tile([128, 2 * FREE], f16)
            red = rpool.tile([128, 2 * W], f16)
            # one scan per tile: fused mask-multiply + segmented reduce
            nc.vector.tensor_tensor_scan(
                out=sc[:],
                data0=sgt[:],
                data1=xt[:],
                initial=0.0,
                op0=AluOpType.mult,
                op1=AluOpType.add,
            )
            # group ends (every 31st elem) * m_end -> the 128 group sums
            ext.tensor_mul(
                out=red[:],
                in0=sc[:].rearrange("p (gw c) -> p gw c", c=CH)[:, :, CH - 1],
                in1=met[:],
            )
            oring.dma_start(
                out=ov[t], in_=red[:].rearrange("p (g w) -> p g w", g=2)
            )


def _emit_dma_floor(tc, x, sg, me, out):
    """DMA-only variant: measures the fp16 HBM roofline (wrong results)."""
    import concourse.mybir as mybir

    nc = tc.nc
    f16 = mybir.dt.float16
    xv = x.rearrange("(t g b) i w c -> t (b i) g (w c)", g=2, b=2)
    ov = out.rearrange("(t g b) i w -> t (b i) g w", g=2, b=2)
    with (
        tc.tile_pool(name="const", bufs=1) as cpool,
        tc.tile_pool(name="xbuf", bufs=4) as xpool,
    ):
        met = cpool.tile([128, 2 * W], f16)
        nc.sync.dma_start(out=met[:], in_=me)
        for t in range(N_TILES):
            xt = xpool.tile([128, 2 * FREE], f16)
            xtv = xt[:].rearrange("p (g f) -> p g f", g=2)
            nc.sync.dma_start(out=xtv[:, 0], in_=xv[t, :, 0])
            nc.sync.dma_start(out=xtv[:, 1], in_=xv[t, :, 1])
            nc.scalar.dma_start(
                out=ov[t], in_=met[:].rearrange("p (g w) -> p g w", g=2)
            )


def build_nc(variant: str = "scan_gp14", reps: int = 1):
    key = (variant, reps)
    if key in _nc_cache:
        return _nc_cache[key]

    import concourse.bacc as bacc
    import concourse.mybir as mybir
    import concourse.tile as tile

    f16 = mybir.dt.float16
    nc = bacc.Bacc("TRN2", debug=False, num_devices=N_CORES)
    x = nc.dram_tensor("x", [B_PER_CORE, H, W, CH], f16, kind="ExternalInput").ap()
    sg = nc.dram_tensor("sg", [128, 2 * FREE], f16, kind="ExternalInput").ap()
    me = nc.dram_tensor("me", [128, 2 * W], f16, kind="ExternalInput").ap()
    out = nc.dram_tensor("out", [B_PER_CORE, H, W], f16, kind="ExternalOutput").ap()

    # variant grammar: "scan[_e{ENG}][_b{BUFS}][_o{RING}][_i{RING}]" | "dma"
    if variant.startswith("scan"):
        kwargs = {}
        for part in variant.split("_")[1:]:
            if part.startswith("e"):
                kwargs["ext_eng"] = part[1:]
            elif part.startswith("b"):
                kwargs["bufs"] = int(part[1:])
            elif part.startswith("o"):
                kwargs["out_ring"] = part[1:]
            elif part.startswith("i"):
                kwargs["in_ring"] = part[1:]
            else:
                raise ValueError(variant)
    elif variant != "dma":
        raise ValueError(variant)

    with tile.TileContext(nc) as tc:
        for _ in range(reps):
            if variant == "dma":
                _emit_dma_floor(tc, x, sg, me, out)
            else:
                _emit_scan(tc, x, sg, me, out, **kwargs)

    nc.compile()
    _nc_cache[key] = nc
    return nc


def host_tensors(w: np.ndarray):
    """sigma [128, 1984] and m_end [128, 128] fp16 tensors from the weights."""
    m32 = np.sign(w.astype(np.float32)).reshape(32, 32)
    i = np.arange(H)[:, None, None]
    wi = np.arange(W)[None, :, None]
    c = np.arange(CH)[None, None, :]
    M = m32[(i - c) % 32, wi % 32]  # [64, 64, 31]
    sig = np.zeros_like(M)
    sig[:, :, 1:] = M[:, :, 1:] * M[:, :, :-1]
    # partition = (b%2, i) -> tile rows x2; free = (g, w, c) -> tile cols x2
    sg = np.tile(sig.reshape(H, FREE), (2, 2)).astype(np.float16)  # [128, 3968]
    me = np.tile(M[:, :, CH - 1], (2, 2)).astype(np.float16)  # [128, 128]
    return sg, me


VARIANT = "scan"


def kernel(x: np.ndarray, w: np.ndarray) -> np.ndarray:
    from concourse.bass_utils import run_bass_kernel_spmd

    x16 = np.ascontiguousarray(np.asarray(x), dtype=np.float16)
    sg, me = host_tensors(np.asarray(w))

    nc = build_nc(VARIANT, 1)
    in_maps = [
        {"x": x16[c * B_PER_CORE : (c + 1) * B_PER_CORE], "sg": sg, "me": me}
        for c in range(N_CORES)
    ]
    res = run_bass_kernel_spmd(nc, in_maps, core_ids=list(range(N_CORES)), trace=TRACE)
    if TRACE and res.exec_time_ns is not None:
        kernel.last_exec_time_ns = res.exec_time_ns
    out = np.concatenate([r["out"] for r in res.results], axis=0)
    return out.astype(np.float32)


kernel.last_exec_time_ns = None
